# revision 66
# baseline (speedup 1.0000x reference)
"""Cosformer attention (causal linear attention with cos reweighting) on 8
Trainium2 NeuronCores.

Sharding: n = bsz*heads = 16 sequences -> 2 per core. Core c handles batch-half
i = c//4 and head-pair p = c%4 (heads 2p, 2p+1). Fully data/head parallel; the
only cross-core interaction is the host-side sum of output-projection partials.

Final layout (~48.5us HW, vs 61.6us baseline). Measured constraints that shaped
it: per-queue DMA bandwidth is only ~40-95 GB/s (three queues: sync HWDGE,
scalar HWDGE, gpsimd SWDGE), each matmul costs ~165ns fixed + 0.41ns/moving
-column at full PE pstate (1.6x slower until ~3us of continuous execution),
and f32r matmuls run in 4-pass fp32_mode=HIGH — hence the all-bf16 datapath.
 - each DMA pays ~2us of queue-serializing completion latency, so inputs are
   consolidated into FIVE large transfers (c1 critical pack, c2 all other
   constants, xt halves) and output partials go out as batched 4-chunk and
   3-chunk stores plus one short tail single.
 - k^T comes from XBAR dma_start_transpose (one per head per half), attn^T
   from per-chunk PE transposes (keeps the out-proj pipeline short).
 - persistent vt pair tiles with preset ones-columns; one strided pair copy
   per chunk instead of per-head copies + memsets.
 - chunks 4-7 are software-pipelined (phase1 = vt/b/intra, phase2 = inter/
   state/normalize, skewed) so chunk c+1's phase1 fills the in-order PE
   queue while chunk c's state matmuls wait on the th1 XBAR k-transposes.
 - qkv, the B matrix, and the running state are all head-PAIR PSUM tiles
   (one bank each; head a's start=True zeroes the whole bank, head b rides
   with start=False), halving their eviction/mask ops and semaphores.
 - 12 warmup matmuls on a memset scratch tile ramp the PE pstate and
   bridge the PE to the first input pack's arrival (~14us: 6.5us fixed
   preamble before the first DMA packet + the c1/xt0 stream time).

Per-core kernel (L=1024 tokens, d=64 per head, pair feature dim P=128):
  1. Feat-major projections per head (duplicated-W trick) -> relu(+bias) ->
     * [sin;cos] row table -> bf16 q_^T,k_^T. V^T projected once per pair.
  2. Chunked causal linear attention (bf16 matmuls, fp32 PSUM), chunk=128:
       B    = masked A^T (upper-tri j<=i)
       qkv  = B.T @ V~  +  q^T.T @ S     (V~ = [V|1]; col 64 = denominator)
       S   += K_tok.T @ V~ in a persistent PSUM bank (fp32, no drift)
       attn = qkv[:,0:64] * 1/max(denom,eps)  -> attn_all (token-major)
  3. per-chunk PE transpose -> bf16 out-proj partial -> batched DRAM stores.
Host sums 4 partials per batch-half in f32, adds bo, reinterleaves rows.
"""

import os
import sys

import numpy as np

for _p in ("/opt/trn_rl_repo", "/root/.axon_site/_ro/trn_rl_repo"):
    if os.path.isdir(_p) and _p not in sys.path:
        sys.path.insert(0, _p)

N_HEAD = 8
E = 512
L = 1024  # sequence length per batch-half
BSZ = 2
D = 64  # head dim
P = 128  # partition/chunk/pair-feature size
NCHUNK = L // P
EPS = 1e-6
N_CORES = 8
TH = 512  # token-half width for projections

# pack layouts (bf16 columns)
# c1: [bias f32-bits (8) | wq_a (512) | wk_a (512)]
_C1_BIAS = 0
_C1_WQA = 8
_C1_WKA = 520
_C1_COLS = 1032
# c2: [wq_b | wk_b | ident | wv | mask | wo | scb]
_C2_WQB = 0
_C2_WKB = 512
_C2_IDENT = 1024
_C2_WV = 1152
_C2_MASK = 1664
_C2_WO = 1920
_C2_SCB = 2432
_C2_COLS = 3456

_CACHE = {}


def _build_bass():
    import concourse.bass as bass
    import concourse.tile as tile
    from concourse import bacc, mybir
    from contextlib import ExitStack

    f32 = mybir.dt.float32
    bf16 = mybir.dt.bfloat16
    AF = mybir.ActivationFunctionType
    D1 = D + 1

    nc = bacc.Bacc("TRN2", target_bir_lowering=False, debug=False)

    # DMA fixed cost is ~2us per transfer (queue-serializing completion
    # receipt), so inputs are consolidated into FOUR large transfers:
    # c1 (bias+wqa+wka, needed first), c2 (everything else constant),
    # xt0/xt1 (token halves, all four e-slices each).
    c1_d = nc.dram_tensor("c1", [P, _C1_COLS], bf16, kind="ExternalInput")
    c2_d = nc.dram_tensor("c2", [P, _C2_COLS], bf16, kind="ExternalInput")
    xt0a_d = nc.dram_tensor("xt0a", [P, 2 * TH], bf16, kind="ExternalInput")
    xt0b_d = nc.dram_tensor("xt0b", [P, 2 * TH], bf16, kind="ExternalInput")
    xt1_d = nc.dram_tensor("xt1", [P, 4 * TH], bf16, kind="ExternalInput")
    out_d = nc.dram_tensor("out", [L, E], bf16, kind="ExternalOutput")

    with tile.TileContext(nc) as tc:
        with ExitStack() as ctx:
            ep = ctx.enter_context
            cpool = ep(tc.tile_pool(name="const", bufs=1))
            seqp = ep(tc.tile_pool(name="seq", bufs=1))
            bp = ep(tc.tile_pool(name="bsb", bufs=4))
            sp = ep(tc.tile_pool(name="state", bufs=4))
            atp = ep(tc.tile_pool(name="attnT", bufs=3))
            outp = ep(tc.tile_pool(name="outsb", bufs=2))
            rp = ep(tc.tile_pool(name="rcol", bufs=4))
            big_ps = ep(tc.tile_pool(name="bigps", bufs=2, space="PSUM"))
            sq_ps = ep(tc.tile_pool(name="sqps", bufs=2, space="PSUM"))
            acc_ps = ep(tc.tile_pool(name="accps", bufs=3, space="PSUM"))
            s_ps = ep(tc.tile_pool(name="sps", bufs=1, space="PSUM"))

            # ---- loads: 4 large transfers (2 per HWDGE ring) ----
            c1_t = cpool.tile([P, _C1_COLS], bf16, name="c1_t")
            nc.sync.dma_start(c1_t[:], c1_d[:, :])
            xt0_t = cpool.tile([P, 4 * TH], bf16, name="xt0_t")
            nc.scalar.dma_start(xt0_t[:, 0 : 2 * TH], xt0a_d[:, :])
            nc.scalar.dma_start(xt0_t[:, 2 * TH : 4 * TH], xt0b_d[:, :])
            c2_t = cpool.tile([P, _C2_COLS], bf16, name="c2_t")
            nc.sync.dma_start(c2_t[:], c2_d[:, :])
            xt1_t = cpool.tile([P, 4 * TH], bf16, name="xt1_t")
            nc.scalar.dma_start(xt1_t[:], xt1_d[:, :])

            # xslc[e][th] -> AP of the e-slice for token-half th
            def xslc(e, th):
                t = xt0_t if th == 0 else xt1_t
                return t[:, e * TH : (e + 1) * TH]

            wt = {}
            wt["wq_a"] = [
                c1_t[:, _C1_WQA + e * P : _C1_WQA + (e + 1) * P] for e in range(4)
            ]
            wt["wk_a"] = [
                c1_t[:, _C1_WKA + e * P : _C1_WKA + (e + 1) * P] for e in range(4)
            ]
            wt["wq_b"] = [
                c2_t[:, _C2_WQB + e * P : _C2_WQB + (e + 1) * P] for e in range(4)
            ]
            wt["wk_b"] = [
                c2_t[:, _C2_WKB + e * P : _C2_WKB + (e + 1) * P] for e in range(4)
            ]
            wt["wv"] = [
                c2_t[:, _C2_WV + e * P : _C2_WV + (e + 1) * P] for e in range(4)
            ]
            mask2_t = c2_t[:, _C2_MASK : _C2_MASK + 256]
            ident_t = c2_t[:, _C2_IDENT : _C2_IDENT + 128]
            wo_t = c2_t[:, _C2_WO : _C2_WO + E]
            scb_sb = c2_t[:, _C2_SCB : _C2_SCB + L]
            bias4 = c1_t[:, _C1_BIAS : _C1_BIAS + 8].bitcast(f32)
            bt = {
                nm: bias4[:, i : i + 1]
                for i, nm in enumerate(("bq_a", "bq_b", "bk_a", "bk_b"))
            }

            # ---- PE warmup: ramp the pstate UNDER the load stream. The
            # scratch tile is memset by gpsimd (no DMA dependency), so the
            # warmups run while c1/xt0 are still in flight.
            warm = cpool.tile([P, TH], bf16, name="warm")
            nc.vector.memset(warm[:], 0.25)
            for w in range(12):
                wp = big_ps.tile([P, TH], f32, tag="big", name=f"warm{w}")
                nc.tensor.matmul(
                    wp[:], warm[:, 0:P], warm[:], start=True, stop=True
                )

            # ---- persistent vt pair tiles: [va | 1 | vb | 1], ones preset ----
            vt2 = [cpool.tile([P, 2 * D1], bf16, name=f"vt2_{i}") for i in range(2)]
            for i in range(2):
                nc.gpsimd.memset(vt2[i][:, D:D1], 1.0)
                nc.gpsimd.memset(vt2[i][:, D1 + D : 2 * D1], 1.0)

            q_seq = {h: seqp.tile([P, L], bf16, name=f"q_{h}") for h in "ab"}
            k_seq = {h: seqp.tile([P, L], bf16, name=f"k_{h}") for h in "ab"}
            v_seq = seqp.tile([P, L], bf16, name="v_pair")
            kt = {h: seqp.tile([P, L], bf16, name=f"kt_{h}") for h in "ab"}
            attn_all = seqp.tile([P, L], bf16, name="attn_all")

            def kt_T(h, th, eng):
                sl = slice(th * TH, (th + 1) * TH)
                dst = kt[h][:, sl].rearrange("p (c m) -> p c m", m=P)
                eng.dma_start_transpose(dst, k_seq[h][:, sl])

            def project_half(seq, wname, bname, outname, th, mul_eng):
                ps = big_ps.tile([P, TH], f32, tag="big", name=f"{outname}_ps{th}")
                for e in range(4):
                    nc.tensor.matmul(
                        ps[:], wt[wname][e], xslc(e, th),
                        start=(e == 0), stop=(e == 3),
                    )
                sl = seq[:, th * TH : (th + 1) * TH]
                if bname is None:
                    nc.scalar.copy(sl, ps[:])
                else:
                    nc.scalar.activation(sl, ps[:], AF.Relu, bias=bt[bname])
                    mul_eng.tensor_mul(sl, sl, scb_sb[:, th * TH : (th + 1) * TH])

            def project_th(th):
                eng = nc.vector if th == 0 else nc.gpsimd
                project_half(q_seq["a"], "wq_a", "bq_a", "q_a", th, eng)
                project_half(k_seq["a"], "wk_a", "bk_a", "k_a", th, eng)
                project_half(q_seq["b"], "wq_b", "bq_b", "q_b", th, eng)
                project_half(k_seq["b"], "wk_b", "bk_b", "k_b", th, eng)
                project_half(v_seq, "wv", None, "v_pair", th, None)

            # ---- attention ----
            s_pair = s_ps.tile([P, 2 * D1], f32, name="s_pair")
            state = {"prev": None}

            # Each chunk splits into phase1 (vt/b/intra — no kt or state
            # dependency) and phase2 (inter/state/normalize), so the second
            # half can software-pipeline: chunk c+1's phase1 fills the PE
            # while chunk c's state waits on the kt XBAR transposes, and the
            # snapshot->inter chain gains a phase of slack.
            pend = {}

            def chunk_p1(c):
                cs = slice(c * P, (c + 1) * P)
                vt_ps = acc_ps.tile([P, P], bf16, tag="acc", name=f"vtps{c}")
                nc.tensor.matmul(vt_ps[:], v_seq[:, cs], ident_t, is_transpose=True)
                vt = vt2[c % 2]
                nc.vector.tensor_copy(
                    vt[:].rearrange("p (b x) -> p b x", x=D1)[:, :, 0:D],
                    vt_ps[:].rearrange("p (b x) -> p b x", x=D),
                )
                vts = {"a": vt[:, 0:D1], "b": vt[:, D1 : 2 * D1]}
                # both heads' B in one PSUM bank (a: start=True zeroes the
                # bank, b rides with start=False) -> ONE 256-wide mask mul
                b_ps = sq_ps.tile([P, 2 * P], f32, tag="sq", name=f"bps{c}")
                nc.tensor.matmul(
                    b_ps[:, 0:P], k_seq["a"][:, cs], q_seq["a"][:, cs],
                    start=True, stop=False, skip_group_check=True,
                )
                nc.tensor.matmul(
                    b_ps[:, P : 2 * P], k_seq["b"][:, cs], q_seq["b"][:, cs],
                    start=False, stop=True, skip_group_check=True,
                )
                b_sb = bp.tile([P, 2 * P], bf16, tag="bsb", name=f"bsb{c}")
                nc.vector.tensor_mul(b_sb[:], b_ps[:], mask2_t)
                bsb = {"a": b_sb[:, 0:P], "b": b_sb[:, P : 2 * P]}
                qkv = acc_ps.tile([P, 2 * D1], f32, tag="acc", name=f"qkv{c}")
                nc.tensor.matmul(
                    qkv[:, 0:D1], bsb["a"], vts["a"],
                    start=True, stop=False, skip_group_check=True,
                )
                nc.tensor.matmul(
                    qkv[:, D1 : 2 * D1], bsb["b"], vts["b"],
                    start=False, stop=(c == 0), skip_group_check=True,
                )
                pend[c] = (qkv, vts)

            def chunk_p2(c):
                cs = slice(c * P, (c + 1) * P)
                qkv, vts = pend.pop(c)
                if c > 0:
                    S = state["prev"]
                    nc.tensor.matmul(
                        qkv[:, 0:D1], q_seq["a"][:, cs], S[:, 0:D1],
                        start=False, stop=False, skip_group_check=True,
                    )
                    nc.tensor.matmul(
                        qkv[:, D1 : 2 * D1], q_seq["b"][:, cs], S[:, D1 : 2 * D1],
                        start=False, stop=True, skip_group_check=True,
                    )
                if c < NCHUNK - 1:
                    for j, h in enumerate("ab"):
                        nc.tensor.matmul(
                            s_pair[:, j * D1 : (j + 1) * D1],
                            kt[h][:, cs],
                            vts[h],
                            start=(c == 0 and j == 0),
                            stop=(c == NCHUNK - 2),
                            skip_group_check=True,
                        )
                    s_new = sp.tile([P, 2 * D1], bf16, tag="S", name=f"S{c}")
                    nc.scalar.copy(s_new[:], s_pair[:])
                    state["prev"] = s_new
                r_col = rp.tile([P, 4], f32, tag="r", name=f"r{c}")
                dens = qkv[:].rearrange("p (h x) -> p h x", x=D1)[:, :, D : D + 1]
                nc.vector.tensor_scalar_max(r_col[:, 0:2], dens, EPS)
                nc.vector.reciprocal(r_col[:, 2:4], r_col[:, 0:2])
                nc.vector.tensor_scalar_mul(
                    attn_all[:, c * P : c * P + D], qkv[:, 0:D], r_col[:, 2:3]
                )
                nc.vector.tensor_scalar_mul(
                    attn_all[:, c * P + D : (c + 1) * P],
                    qkv[:, D1 : D1 + D],
                    r_col[:, 3:4],
                )

            def attn_chunk(c):
                chunk_p1(c)
                chunk_p2(c)

            # chunk outputs collect into two 4-chunk group buffers; each
            # group goes out as ONE large store (chunks 4-6 batched + a
            # short single for chunk 7 so the tail stays small).
            o_buf = [
                outp.tile([P, 4 * E], bf16, tag="osb", name=f"obuf{g}")
                for g in range(2)
            ]

            def batch_store(eng, lo, hi, g):
                dst = out_d[lo * P : hi * P, :].rearrange(
                    "(b r) e -> r b e", b=hi - lo
                )
                src = o_buf[g][:, (lo % 4) * E : (hi - 4 * g) * E].rearrange(
                    "p (b e) -> p b e", e=E
                )
                eng.dma_start(dst, src)

            def outproj(c):
                at_ps = acc_ps.tile([P, P], bf16, tag="acc", name=f"atps{c}")
                nc.tensor.matmul(
                    at_ps[:], attn_all[:, c * P : (c + 1) * P], ident_t,
                    is_transpose=True,
                )
                at_sb = atp.tile([P, P], bf16, tag="at", name=f"at{c}")
                if c % 2 == 0:
                    nc.scalar.copy(at_sb[:], at_ps[:])
                else:
                    nc.vector.tensor_copy(at_sb[:], at_ps[:])
                o_ps = big_ps.tile([P, E], f32, tag="big", name=f"ops{c}")
                nc.tensor.matmul(o_ps[:], at_sb[:], wo_t, start=True, stop=True)
                dst = o_buf[c // 4][:, (c % 4) * E : (c % 4 + 1) * E]
                if c % 2 == 0:
                    nc.scalar.copy(dst, o_ps[:])
                else:
                    nc.vector.tensor_copy(dst, o_ps[:])
                if c == 3:
                    batch_store(nc.gpsimd, 0, 4, 0)
                elif c == 6:
                    batch_store(nc.gpsimd, 4, 7, 1)
                elif c == 7:
                    batch_store(nc.sync, 7, 8, 1)

            project_th(0)
            kt_T("a", 0, nc.sync)
            kt_T("b", 0, nc.scalar)
            attn_chunk(0)
            attn_chunk(1)
            outproj(0)
            attn_chunk(2)
            outproj(1)
            attn_chunk(3)
            outproj(2)
            project_th(1)
            kt_T("a", 1, nc.sync)
            kt_T("b", 1, nc.scalar)
            outproj(3)
            chunk_p1(4)
            chunk_p1(5)
            chunk_p2(4)
            chunk_p1(6)
            chunk_p2(5)
            outproj(4)
            chunk_p1(7)
            chunk_p2(6)
            outproj(5)
            chunk_p2(7)
            outproj(6)
            outproj(7)

    nc.compile()
    return nc


def _get_nc():
    if "nc" not in _CACHE:
        _CACHE["nc"] = _build_bass()
    return _CACHE["nc"]


def make_in_maps(query, Wq, bq, Wk, bk, Wv, bv, Wo, bo):
    import ml_dtypes

    f32 = np.float32
    bf16 = ml_dtypes.bfloat16
    query = np.asarray(query, f32)
    x3 = query.reshape(L, BSZ, E)  # faithful torch .view reshape
    idx = (np.pi / 2) * np.arange(1, L + 1, dtype=f32) / f32(L)
    sinv = np.sin(idx).astype(f32)
    cosv = np.cos(idx).astype(f32)

    Wq, Wk, Wv, Wo = (np.asarray(w, f32) for w in (Wq, Wk, Wv, Wo))
    bq, bk, bv = (np.asarray(b, f32) for b in (bq, bk, bv))

    def wslice_dup(W, h):
        """(128, 512): [Wh.T | Wh.T] dup cols laid out as 4 e-tiles of 128."""
        w = W[D * h : D * (h + 1), :].T  # (512, 64)
        wd = np.concatenate([w, w], axis=1)  # (512, 128)
        return np.hstack([wd[e * P : (e + 1) * P, :] for e in range(4)])

    def bdup(b, h):
        bb = b[D * h : D * (h + 1)]
        return np.concatenate([bb, bb]).astype(f32)

    ident = np.eye(P, dtype=bf16)
    scb = np.empty((P, L), f32)
    scb[0:D] = sinv[None, :]
    scb[D:P] = cosv[None, :]
    mask = np.triu(np.ones((P, P), f32)).astype(bf16)

    in_maps = []
    for c in range(N_CORES):
        i, p = divmod(c, 4)
        hA, hB = 2 * p, 2 * p + 1

        biases = np.stack(
            [bdup(bq, hA), bdup(bq, hB), bdup(bk, hA), bdup(bk, hB)], axis=1
        ).astype(f32)  # (128, 4)
        bias_bits = np.ascontiguousarray(biases).view(bf16)  # (128, 8)
        c1 = np.hstack(
            [bias_bits, wslice_dup(Wq, hA).astype(bf16), wslice_dup(Wk, hA).astype(bf16)]
        )
        assert c1.shape == (P, _C1_COLS), c1.shape

        wv_p = Wv[P * p : P * (p + 1), :].T  # (512, 128)
        wv_pack = np.hstack([wv_p[e * P : (e + 1) * P, :] for e in range(4)])
        wo_pack = Wo[:, P * p : P * (p + 1)].T.astype(bf16)  # (128, 512)
        c2 = np.hstack(
            [
                wslice_dup(Wq, hB).astype(bf16),
                wslice_dup(Wk, hB).astype(bf16),
                ident,
                wv_pack.astype(bf16),
                mask,
                mask,
                wo_pack,
                scb.astype(bf16),
            ]
        )
        assert c2.shape == (P, _C2_COLS), c2.shape

        xt = np.ascontiguousarray(x3[:, i, :].T).astype(bf16)  # (512, 1024)
        xt0 = np.hstack([xt[e * P : (e + 1) * P, 0:TH] for e in range(4)])
        xt1 = np.hstack([xt[e * P : (e + 1) * P, TH:L] for e in range(4)])

        in_maps.append(
            dict(
                c1=np.ascontiguousarray(c1),
                c2=np.ascontiguousarray(c2),
                xt0a=np.ascontiguousarray(xt0[:, 0 : 2 * TH]),
                xt0b=np.ascontiguousarray(xt0[:, 2 * TH : 4 * TH]),
                xt1=np.ascontiguousarray(xt1),
            )
        )
    return in_maps


def assemble(partials, bo, bv, Wo):
    out_flat = np.zeros((BSZ * L, E), np.float32)
    ps = [np.asarray(p, np.float32) for p in partials]
    out_flat[0::2] = ps[0] + ps[1] + ps[2] + ps[3]
    out_flat[1::2] = ps[4] + ps[5] + ps[6] + ps[7]
    # V-bias passes through the normalized attention additively (exact up to
    # the eps clip): attn(v + bv) = attn(v) + bv, so fold bv @ Wo.T into bo.
    bo_eff = np.asarray(bo, np.float32) + np.asarray(bv, np.float32) @ np.asarray(
        Wo, np.float32
    ).T.astype(np.float32)
    out_flat += bo_eff[None, :]
    return out_flat.reshape(BSZ, L, E)


def run(inputs, trace=False):
    from concourse.bass_utils import run_bass_kernel_spmd

    in_maps = make_in_maps(**inputs)
    nc = _get_nc()
    res = run_bass_kernel_spmd(nc, in_maps, list(range(N_CORES)), trace=trace)
    partials = [r["out"] for r in res.results]
    return assemble(partials, inputs["bo"], inputs["bv"], inputs["Wo"]), res


def kernel(**inputs):
    out, _ = run(inputs, trace=False)
    return out


# revision 67
# speedup vs baseline: 1.1058x; 1.1058x over previous
"""Cosformer attention (causal linear attention with cos reweighting) on 8
Trainium2 NeuronCores.

Sharding: n = bsz*heads = 16 sequences -> 2 per core. Core c handles batch-half
i = c//4 and head-pair p = c%4 (heads 2p, 2p+1). Fully data/head parallel; the
only cross-core interaction is the host-side sum of output-projection partials.

Final layout (~48.5us HW, vs 61.6us baseline). Measured constraints that shaped
it: per-queue DMA bandwidth is only ~40-95 GB/s (three queues: sync HWDGE,
scalar HWDGE, gpsimd SWDGE), each matmul costs ~165ns fixed + 0.41ns/moving
-column at full PE pstate (1.6x slower until ~3us of continuous execution),
and f32r matmuls run in 4-pass fp32_mode=HIGH — hence the all-bf16 datapath.
 - each DMA pays ~2us of queue-serializing completion latency, so inputs are
   consolidated into FIVE large transfers (c1 critical pack, c2 all other
   constants, xt halves) and output partials go out as batched 4-chunk and
   3-chunk stores plus one short tail single.
 - k^T comes from XBAR dma_start_transpose (one per head per half), attn^T
   from per-chunk PE transposes (keeps the out-proj pipeline short).
 - persistent vt pair tiles with preset ones-columns; one strided pair copy
   per chunk instead of per-head copies + memsets.
 - chunks 4-7 are software-pipelined (phase1 = vt/b/intra, phase2 = inter/
   state/normalize, skewed) so chunk c+1's phase1 fills the in-order PE
   queue while chunk c's state matmuls wait on the th1 XBAR k-transposes.
 - qkv, the B matrix, and the running state are all head-PAIR PSUM tiles
   (one bank each; head a's start=True zeroes the whole bank, head b rides
   with start=False), halving their eviction/mask ops and semaphores.
 - 12 warmup matmuls on a memset scratch tile ramp the PE pstate and
   bridge the PE to the first input pack's arrival (~14us: 6.5us fixed
   preamble before the first DMA packet + the c1/xt0 stream time).

Per-core kernel (L=1024 tokens, d=64 per head, pair feature dim P=128):
  1. Feat-major projections per head (duplicated-W trick) -> relu(+bias) ->
     * [sin;cos] row table -> bf16 q_^T,k_^T. V^T projected once per pair.
  2. Chunked causal linear attention (bf16 matmuls, fp32 PSUM), chunk=128:
       B    = masked A^T (upper-tri j<=i)
       qkv  = B.T @ V~  +  q^T.T @ S     (V~ = [V|1]; col 64 = denominator)
       S   += K_tok.T @ V~ in a persistent PSUM bank (fp32, no drift)
       attn = qkv[:,0:64] * 1/max(denom,eps)  -> attn_all (token-major)
  3. per-chunk PE transpose -> bf16 out-proj partial -> batched DRAM stores.
Host sums 4 partials per batch-half in f32, adds bo, reinterleaves rows.
"""

import os
import sys

import numpy as np

for _p in ("/opt/trn_rl_repo", "/root/.axon_site/_ro/trn_rl_repo"):
    if os.path.isdir(_p) and _p not in sys.path:
        sys.path.insert(0, _p)

N_HEAD = 8
E = 512
L = 1024  # sequence length per batch-half
BSZ = 2
D = 64  # head dim
P = 128  # partition/chunk/pair-feature size
NCHUNK = L // P
EPS = 1e-6
N_CORES = 8
TH = 512  # token-half width for projections

# pack layouts (bf16 columns)
# c1: [bias f32-bits (8) | wq_a (512) | wk_a (512)]
_C1_BIAS = 0
_C1_WQA = 8
_C1_WKA = 520
_C1_COLS = 1032
# c2: [wq_b | wk_b | ident | wv | mask | wo | scb]
_C2_WQB = 0
_C2_WKB = 512
_C2_IDENT = 1024
_C2_WV = 1152
_C2_MASK = 1664
_C2_WO = 1920
_C2_SCB = 2432
_C2_COLS = 3456

_CACHE = {}


def _build_bass():
    import concourse.bass as bass
    import concourse.tile as tile
    from concourse import bacc, mybir
    from contextlib import ExitStack

    f32 = mybir.dt.float32
    bf16 = mybir.dt.bfloat16
    AF = mybir.ActivationFunctionType
    D1 = D + 1

    nc = bacc.Bacc("TRN2", target_bir_lowering=False, debug=False)

    # DMA fixed cost is ~2us per transfer (queue-serializing completion
    # receipt), so inputs are consolidated into FOUR large transfers:
    # c1 (bias+wqa+wka, needed first), c2 (everything else constant),
    # xt0/xt1 (token halves, all four e-slices each).
    c1_d = nc.dram_tensor("c1", [P, _C1_COLS], bf16, kind="ExternalInput")
    c2_d = nc.dram_tensor("c2", [P, _C2_COLS], bf16, kind="ExternalInput")
    xt0a_d = nc.dram_tensor("xt0a", [P, 2 * TH], bf16, kind="ExternalInput")
    xt0b_d = nc.dram_tensor("xt0b", [P, 2 * TH], bf16, kind="ExternalInput")
    xt1_d = nc.dram_tensor("xt1", [P, 4 * TH], bf16, kind="ExternalInput")
    out_d = nc.dram_tensor("out", [L, E], bf16, kind="ExternalOutput")

    with tile.TileContext(nc) as tc:
        with ExitStack() as ctx:
            ep = ctx.enter_context
            cpool = ep(tc.tile_pool(name="const", bufs=1))
            seqp = ep(tc.tile_pool(name="seq", bufs=1))
            bp = ep(tc.tile_pool(name="bsb", bufs=4))
            sp = ep(tc.tile_pool(name="state", bufs=4))
            atp = ep(tc.tile_pool(name="attnT", bufs=3))
            outp = ep(tc.tile_pool(name="outsb", bufs=2))
            rp = ep(tc.tile_pool(name="rcol", bufs=4))
            big_ps = ep(tc.tile_pool(name="bigps", bufs=2, space="PSUM"))
            sq_ps = ep(tc.tile_pool(name="sqps", bufs=2, space="PSUM"))
            acc_ps = ep(tc.tile_pool(name="accps", bufs=3, space="PSUM"))
            s_ps = ep(tc.tile_pool(name="sps", bufs=1, space="PSUM"))

            # ---- loads: 4 large transfers (2 per HWDGE ring) ----
            c1_t = cpool.tile([P, _C1_COLS], bf16, name="c1_t")
            nc.sync.dma_start(c1_t[:], c1_d[:, :])
            xt0_t = cpool.tile([P, 4 * TH], bf16, name="xt0_t")
            nc.scalar.dma_start(xt0_t[:, 0 : 2 * TH], xt0a_d[:, :])
            nc.scalar.dma_start(xt0_t[:, 2 * TH : 4 * TH], xt0b_d[:, :])
            c2_t = cpool.tile([P, _C2_COLS], bf16, name="c2_t")
            nc.sync.dma_start(c2_t[:], c2_d[:, :])
            xt1_t = cpool.tile([P, 4 * TH], bf16, name="xt1_t")
            nc.scalar.dma_start(xt1_t[:], xt1_d[:, :])

            # xslc[e][th] -> AP of the e-slice for token-half th
            def xslc(e, th):
                t = xt0_t if th == 0 else xt1_t
                return t[:, e * TH : (e + 1) * TH]

            wt = {}
            wt["wq_a"] = [
                c1_t[:, _C1_WQA + e * P : _C1_WQA + (e + 1) * P] for e in range(4)
            ]
            wt["wk_a"] = [
                c1_t[:, _C1_WKA + e * P : _C1_WKA + (e + 1) * P] for e in range(4)
            ]
            wt["wq_b"] = [
                c2_t[:, _C2_WQB + e * P : _C2_WQB + (e + 1) * P] for e in range(4)
            ]
            wt["wk_b"] = [
                c2_t[:, _C2_WKB + e * P : _C2_WKB + (e + 1) * P] for e in range(4)
            ]
            wt["wv"] = [
                c2_t[:, _C2_WV + e * P : _C2_WV + (e + 1) * P] for e in range(4)
            ]
            mask2_t = c2_t[:, _C2_MASK : _C2_MASK + 256]
            ident_t = c2_t[:, _C2_IDENT : _C2_IDENT + 128]
            wo_t = c2_t[:, _C2_WO : _C2_WO + E]
            scb_sb = c2_t[:, _C2_SCB : _C2_SCB + L]
            bias4 = c1_t[:, _C1_BIAS : _C1_BIAS + 8].bitcast(f32)
            bt = {
                nm: bias4[:, i : i + 1]
                for i, nm in enumerate(("bq_a", "bq_b", "bk_a", "bk_b"))
            }

            # ---- PE warmup: ramp the pstate UNDER the load stream. The
            # scratch tile is memset by gpsimd (no DMA dependency), so the
            # warmups run while c1/xt0 are still in flight.
            warm = cpool.tile([P, TH], bf16, name="warm")
            nc.vector.memset(warm[:], 0.25)
            for w in range(12):
                wp = big_ps.tile([P, TH], f32, tag="big", name=f"warm{w}")
                nc.tensor.matmul(
                    wp[:], warm[:, 0:P], warm[:], start=True, stop=True
                )

            # ---- persistent vt pair tiles: [va | 1 | vb | 1], ones preset ----
            vt2 = [cpool.tile([P, 2 * D1], bf16, name=f"vt2_{i}") for i in range(2)]
            for i in range(2):
                nc.gpsimd.memset(vt2[i][:, D:D1], 1.0)
                nc.gpsimd.memset(vt2[i][:, D1 + D : 2 * D1], 1.0)

            q_seq = {h: seqp.tile([P, L], bf16, name=f"q_{h}") for h in "ab"}
            k_seq = {h: seqp.tile([P, L], bf16, name=f"k_{h}") for h in "ab"}
            v_seq = seqp.tile([P, L], bf16, name="v_pair")
            kt = {h: seqp.tile([P, L], bf16, name=f"kt_{h}") for h in "ab"}
            attn_all = seqp.tile([P, L], bf16, name="attn_all")

            def kt_T(h, th, eng):
                sl = slice(th * TH, (th + 1) * TH)
                dst = kt[h][:, sl].rearrange("p (c m) -> p c m", m=P)
                eng.dma_start_transpose(dst, k_seq[h][:, sl])

            def project_half(seq, wname, bname, outname, th, mul_eng):
                ps = big_ps.tile([P, TH], f32, tag="big", name=f"{outname}_ps{th}")
                for e in range(4):
                    nc.tensor.matmul(
                        ps[:], wt[wname][e], xslc(e, th),
                        start=(e == 0), stop=(e == 3),
                    )
                sl = seq[:, th * TH : (th + 1) * TH]
                if bname is None:
                    nc.scalar.copy(sl, ps[:])
                else:
                    nc.scalar.activation(sl, ps[:], AF.Relu, bias=bt[bname])
                    mul_eng.tensor_mul(sl, sl, scb_sb[:, th * TH : (th + 1) * TH])

            def project_th(th):
                eng = nc.vector if th == 0 else nc.gpsimd
                project_half(q_seq["a"], "wq_a", "bq_a", "q_a", th, eng)
                project_half(k_seq["a"], "wk_a", "bk_a", "k_a", th, eng)
                project_half(q_seq["b"], "wq_b", "bq_b", "q_b", th, eng)
                project_half(k_seq["b"], "wk_b", "bk_b", "k_b", th, eng)
                project_half(v_seq, "wv", None, "v_pair", th, None)

            # ---- attention ----
            s_pair = s_ps.tile([P, 2 * D1], f32, name="s_pair")
            state = {"prev": None}

            # Each chunk splits into phase1 (vt/b/intra — no kt or state
            # dependency) and phase2 (inter/state/normalize), so the second
            # half can software-pipeline: chunk c+1's phase1 fills the PE
            # while chunk c's state waits on the kt XBAR transposes, and the
            # snapshot->inter chain gains a phase of slack.
            pend = {}

            def chunk_p1(c):
                cs = slice(c * P, (c + 1) * P)
                vt_ps = acc_ps.tile([P, P], bf16, tag="acc", name=f"vtps{c}")
                nc.tensor.matmul(vt_ps[:], v_seq[:, cs], ident_t, is_transpose=True)
                vt = vt2[c % 2]
                nc.vector.tensor_copy(
                    vt[:].rearrange("p (b x) -> p b x", x=D1)[:, :, 0:D],
                    vt_ps[:].rearrange("p (b x) -> p b x", x=D),
                )
                vts = {"a": vt[:, 0:D1], "b": vt[:, D1 : 2 * D1]}
                # both heads' B in one PSUM bank (a: start=True zeroes the
                # bank, b rides with start=False) -> ONE 256-wide mask mul
                b_ps = sq_ps.tile([P, 2 * P], f32, tag="sq", name=f"bps{c}")
                nc.tensor.matmul(
                    b_ps[:, 0:P], k_seq["a"][:, cs], q_seq["a"][:, cs],
                    start=True, stop=False, skip_group_check=True,
                )
                nc.tensor.matmul(
                    b_ps[:, P : 2 * P], k_seq["b"][:, cs], q_seq["b"][:, cs],
                    start=False, stop=True, skip_group_check=True,
                )
                b_sb = bp.tile([P, 2 * P], bf16, tag="bsb", name=f"bsb{c}")
                nc.vector.tensor_mul(b_sb[:], b_ps[:], mask2_t)
                bsb = {"a": b_sb[:, 0:P], "b": b_sb[:, P : 2 * P]}
                qkv = acc_ps.tile([P, 2 * D1], f32, tag="acc", name=f"qkv{c}")
                nc.tensor.matmul(
                    qkv[:, 0:D1], bsb["a"], vts["a"],
                    start=True, stop=False, skip_group_check=True,
                )
                nc.tensor.matmul(
                    qkv[:, D1 : 2 * D1], bsb["b"], vts["b"],
                    start=False, stop=(c == 0), skip_group_check=True,
                )
                pend[c] = (qkv, vts)

            def chunk_p2(c):
                cs = slice(c * P, (c + 1) * P)
                qkv, vts = pend.pop(c)
                if c > 0:
                    S = state["prev"]
                    nc.tensor.matmul(
                        qkv[:, 0:D1], q_seq["a"][:, cs], S[:, 0:D1],
                        start=False, stop=False, skip_group_check=True,
                    )
                    nc.tensor.matmul(
                        qkv[:, D1 : 2 * D1], q_seq["b"][:, cs], S[:, D1 : 2 * D1],
                        start=False, stop=True, skip_group_check=True,
                    )
                if c < NCHUNK - 1:
                    for j, h in enumerate("ab"):
                        nc.tensor.matmul(
                            s_pair[:, j * D1 : (j + 1) * D1],
                            kt[h][:, cs],
                            vts[h],
                            start=(c == 0 and j == 0),
                            stop=(c == NCHUNK - 2),
                            skip_group_check=True,
                        )
                    s_new = sp.tile([P, 2 * D1], bf16, tag="S", name=f"S{c}")
                    nc.scalar.copy(s_new[:], s_pair[:])
                    state["prev"] = s_new
                r_col = rp.tile([P, 4], f32, tag="r", name=f"r{c}")
                dens = qkv[:].rearrange("p (h x) -> p h x", x=D1)[:, :, D : D + 1]
                nc.vector.tensor_scalar_max(r_col[:, 0:2], dens, EPS)
                nc.vector.reciprocal(r_col[:, 2:4], r_col[:, 0:2])
                nc.vector.tensor_scalar_mul(
                    attn_all[:, c * P : c * P + D], qkv[:, 0:D], r_col[:, 2:3]
                )
                nc.vector.tensor_scalar_mul(
                    attn_all[:, c * P + D : (c + 1) * P],
                    qkv[:, D1 : D1 + D],
                    r_col[:, 3:4],
                )

            def attn_chunk(c):
                chunk_p1(c)
                chunk_p2(c)

            # chunk outputs collect into two 4-chunk group buffers; each
            # group goes out as ONE large store (chunks 4-6 batched + a
            # short single for chunk 7 so the tail stays small).
            o_buf = [
                outp.tile([P, 4 * E], bf16, tag="osb", name=f"obuf{g}")
                for g in range(2)
            ]

            def batch_store(eng, lo, hi, g):
                dst = out_d[lo * P : hi * P, :].rearrange(
                    "(b r) e -> r b e", b=hi - lo
                )
                src = o_buf[g][:, (lo % 4) * E : (hi - 4 * g) * E].rearrange(
                    "p (b e) -> p b e", e=E
                )
                eng.dma_start(dst, src)

            def outproj(c):
                at_ps = acc_ps.tile([P, P], bf16, tag="acc", name=f"atps{c}")
                nc.tensor.matmul(
                    at_ps[:], attn_all[:, c * P : (c + 1) * P], ident_t,
                    is_transpose=True,
                )
                at_sb = atp.tile([P, P], bf16, tag="at", name=f"at{c}")
                nc.scalar.copy(at_sb[:], at_ps[:])
                o_ps = big_ps.tile([P, E], f32, tag="big", name=f"ops{c}")
                nc.tensor.matmul(o_ps[:], at_sb[:], wo_t, start=True, stop=True)
                dst = o_buf[c // 4][:, (c % 4) * E : (c % 4 + 1) * E]
                if c % 2 == 0:
                    nc.scalar.copy(dst, o_ps[:])
                else:
                    nc.vector.tensor_copy(dst, o_ps[:])
                if c == 3:
                    batch_store(nc.gpsimd, 0, 4, 0)
                elif c == 6:
                    batch_store(nc.gpsimd, 4, 7, 1)
                elif c == 7:
                    batch_store(nc.sync, 7, 8, 1)

            project_th(0)
            kt_T("a", 0, nc.sync)
            kt_T("b", 0, nc.scalar)
            attn_chunk(0)
            attn_chunk(1)
            outproj(0)
            attn_chunk(2)
            outproj(1)
            attn_chunk(3)
            outproj(2)
            project_th(1)
            kt_T("a", 1, nc.sync)
            kt_T("b", 1, nc.scalar)
            outproj(3)
            chunk_p1(4)
            chunk_p1(5)
            chunk_p2(4)
            chunk_p1(6)
            chunk_p2(5)
            outproj(4)
            chunk_p1(7)
            chunk_p2(6)
            outproj(5)
            chunk_p2(7)
            outproj(6)
            outproj(7)

    nc.compile()
    return nc


def _get_nc():
    if "nc" not in _CACHE:
        _CACHE["nc"] = _build_bass()
    return _CACHE["nc"]


def make_in_maps(query, Wq, bq, Wk, bk, Wv, bv, Wo, bo):
    import ml_dtypes

    f32 = np.float32
    bf16 = ml_dtypes.bfloat16
    query = np.asarray(query, f32)
    x3 = query.reshape(L, BSZ, E)  # faithful torch .view reshape
    idx = (np.pi / 2) * np.arange(1, L + 1, dtype=f32) / f32(L)
    sinv = np.sin(idx).astype(f32)
    cosv = np.cos(idx).astype(f32)

    Wq, Wk, Wv, Wo = (np.asarray(w, f32) for w in (Wq, Wk, Wv, Wo))
    bq, bk, bv = (np.asarray(b, f32) for b in (bq, bk, bv))

    def wslice_dup(W, h):
        """(128, 512): [Wh.T | Wh.T] dup cols laid out as 4 e-tiles of 128."""
        w = W[D * h : D * (h + 1), :].T  # (512, 64)
        wd = np.concatenate([w, w], axis=1)  # (512, 128)
        return np.hstack([wd[e * P : (e + 1) * P, :] for e in range(4)])

    def bdup(b, h):
        bb = b[D * h : D * (h + 1)]
        return np.concatenate([bb, bb]).astype(f32)

    ident = np.eye(P, dtype=bf16)
    scb = np.empty((P, L), f32)
    scb[0:D] = sinv[None, :]
    scb[D:P] = cosv[None, :]
    mask = np.triu(np.ones((P, P), f32)).astype(bf16)

    in_maps = []
    for c in range(N_CORES):
        i, p = divmod(c, 4)
        hA, hB = 2 * p, 2 * p + 1

        biases = np.stack(
            [bdup(bq, hA), bdup(bq, hB), bdup(bk, hA), bdup(bk, hB)], axis=1
        ).astype(f32)  # (128, 4)
        bias_bits = np.ascontiguousarray(biases).view(bf16)  # (128, 8)
        c1 = np.hstack(
            [bias_bits, wslice_dup(Wq, hA).astype(bf16), wslice_dup(Wk, hA).astype(bf16)]
        )
        assert c1.shape == (P, _C1_COLS), c1.shape

        wv_p = Wv[P * p : P * (p + 1), :].T  # (512, 128)
        wv_pack = np.hstack([wv_p[e * P : (e + 1) * P, :] for e in range(4)])
        wo_pack = Wo[:, P * p : P * (p + 1)].T.astype(bf16)  # (128, 512)
        c2 = np.hstack(
            [
                wslice_dup(Wq, hB).astype(bf16),
                wslice_dup(Wk, hB).astype(bf16),
                ident,
                wv_pack.astype(bf16),
                mask,
                mask,
                wo_pack,
                scb.astype(bf16),
            ]
        )
        assert c2.shape == (P, _C2_COLS), c2.shape

        xt = np.ascontiguousarray(x3[:, i, :].T).astype(bf16)  # (512, 1024)
        xt0 = np.hstack([xt[e * P : (e + 1) * P, 0:TH] for e in range(4)])
        xt1 = np.hstack([xt[e * P : (e + 1) * P, TH:L] for e in range(4)])

        in_maps.append(
            dict(
                c1=np.ascontiguousarray(c1),
                c2=np.ascontiguousarray(c2),
                xt0a=np.ascontiguousarray(xt0[:, 0 : 2 * TH]),
                xt0b=np.ascontiguousarray(xt0[:, 2 * TH : 4 * TH]),
                xt1=np.ascontiguousarray(xt1),
            )
        )
    return in_maps


def assemble(partials, bo, bv, Wo):
    out_flat = np.zeros((BSZ * L, E), np.float32)
    ps = [np.asarray(p, np.float32) for p in partials]
    out_flat[0::2] = ps[0] + ps[1] + ps[2] + ps[3]
    out_flat[1::2] = ps[4] + ps[5] + ps[6] + ps[7]
    # V-bias passes through the normalized attention additively (exact up to
    # the eps clip): attn(v + bv) = attn(v) + bv, so fold bv @ Wo.T into bo.
    bo_eff = np.asarray(bo, np.float32) + np.asarray(bv, np.float32) @ np.asarray(
        Wo, np.float32
    ).T.astype(np.float32)
    out_flat += bo_eff[None, :]
    return out_flat.reshape(BSZ, L, E)


def run(inputs, trace=False):
    from concourse.bass_utils import run_bass_kernel_spmd

    in_maps = make_in_maps(**inputs)
    nc = _get_nc()
    res = run_bass_kernel_spmd(nc, in_maps, list(range(N_CORES)), trace=trace)
    partials = [r["out"] for r in res.results]
    return assemble(partials, inputs["bo"], inputs["bv"], inputs["Wo"]), res


def kernel(**inputs):
    out, _ = run(inputs, trace=False)
    return out
